# revision 35
# baseline (speedup 1.0000x reference)
"""Trainium2 Bass kernel for nn_BlockDiagonalLayer.

Computes out[b, n*64+j] = sin(omega[n] * (sum_i x[b,n,i] * W[n,j,i] + bias[n,j]))
for B=2048, N=1024 networks, D_IN=D_OUT=64, sharded over 8 NeuronCores along N.

Per core (128 networks = 64 pairs), per pair p (whole batch B=2048 in one
PSUM tile of 4 banks):
  - PE 3 passes (fp16, 1 cyc/col): f' = wh@xh + wh2@xl8 + wl@xh
      wh  = fp16(W'),  W' = W^T * (omega/2pi)  (block-diag per pair)
      wh2 = fp16(wh * 2^-11)  (exact exponent shift)
      wl  = fp16(W' - wh)
      xh  = fp16(x^T), xl8 = fp8e3m4((x - xh) * 2^11)  [mixed-dtype matmul]
    -> arg error ~5e-4 rad max (Monte Carlo, omega=31).
  - Range reduction sin(2pi*(f' + bsc)) with bsc = (omega/2pi)*bias, two
    engine-balanced chain variants (2/8 pairs ACT-chain, 6/8 DVE-chain;
    PE is the binding engine at ~214us busy so the ACT-chain's extra negI
    matmul pass is rationed):
    ACT-chain (2 ACT ops + 1 extra PE pass, no DVE):
      t2  = fp16(v + (bsc + 1536))       ACT Identity, bias AP; fp16 ulp=1 on
                                         [1024,2048) rounds to EXACTLY k+1536,
                                         k = round(f'+bsc)
      v  += (-I) @ t2                    PE accumulate: v = f'-k-1536
      out = Sin(2pi*v + 2pi*(bsc+1536))  ACT, bias AP; arg = 2pi*(g-k) in ±pi
    DVE-chain (2 DVE ops + 1 ACT op):
      t   = (v + bsc) + MAGIC            DVE tensor_scalar 2-op (AP + imm);
                                         rounds k=round(g) into mantissa
      q   = (t - MAGIC) - v  (= k - f')  DVE scalar_tensor_tensor, exact
      out = Sin(-2pi*q + 2pi*bsc)        ACT, bias AP; arg in ±pi
    (An "AD" variant doing the t2 rounding on DVE exists behind K_ADN.)
  - Output fp8e3m4 (quantization 0.0156; measured total rel err 1.661e-2,
    deterministic for the seed-0 harness inputs; budget 2e-2).
  - One-tile software pipelining (PEND=2) of each tile's final ops keeps the
    in-order PE queue fed across the t2->negI dependency; weight preload
    rides the scalar HWDGE ring so x streaming starts immediately.
No bias matmul (bias rides the rounding-trick biases + Sin bias APs).
Host does layout-only transforms (transpose / block-diag packing / scalars).
HW-measured: ~295 us vs 350-404 us for the previous fp16-hi/lo baseline.
Notes: DVE `mod`, f32r-stationary x fp16-moving matmuls, and 2-bank matmul
outputs are all rejected by the neuronxcc ISA checks (W32R path kept for
reference but disabled).
"""

import numpy as np
import ml_dtypes

import concourse.bass as bass
import concourse.tile as tile
from concourse import bacc, mybir
from concourse.alu_op_type import AluOpType
from concourse.bass_utils import run_bass_kernel_spmd

B, N, D = 2048, 1024, 64
NCORES = 8
NS = N // NCORES          # 128 nets per core
PAIRS = NS // 2           # 64
import os
MMW = int(os.environ.get("K_MMW", "512"))  # matmul moving free dim
EW = int(os.environ.get("K_EW", "1024"))  # elementwise tile width
PSUM_BUFS = 8 // (EW // 512)              # PSUM banks are 512 fp32
T_BUFS = 6 if EW <= 1024 else 3           # t/q/t2 pool depth
PB = int(os.environ.get("K_PB", "2"))     # pairs per x/y DMA transfer
XB, OB = (4, 3) if PB <= 2 else (3, 2)     # stream pool depths (SBUF fit)

TWO_PI = float(2.0 * np.pi)
INV_2PI = float(1.0 / (2.0 * np.pi))
MAGIC = float(1.5 * 2 ** 23)
C1536 = 1536.0

F32 = mybir.dt.float32
F32R = mybir.dt.float32r
FP16 = mybir.dt.float16
BF16 = mybir.dt.bfloat16
F8E3 = mybir.dt.float8e3

OUT_F8 = True             # fp8e3m4 output (else bf16)
# W32R: weights as f32r (nearest-FP22 pre-rounded on host) -> 2 matmul
# passes (w@xh + (w*2^-11)@xl8) instead of 3 fp16 passes.
W32R = os.environ.get("K_W32R", "0") == "1"
WSTRIDE = 256 if W32R else 384
W_DT = F32R if W32R else FP16
ACT_DEN = 16 if W32R else 8
ACT_CHAIN_NUM = int(os.environ.get("K_ACTN", "7" if W32R else "2"))
AD_CHAIN_NUM = int(os.environ.get("K_ADN", "0"))   # AD-chain pairs per DEN
OUT_DT = F8E3 if OUT_F8 else BF16
OUT_NP = ml_dtypes.float8_e3m4 if OUT_F8 else ml_dtypes.bfloat16

# chain pattern over ACT_DEN pairs: "A" = t2 on ACT + negI + Sin;
# "AD" = t2 on DVE + negI + Sin; "D" = magic on DVE + Sin.
_pat = ["D"] * ACT_DEN
for _i in range(ACT_DEN):
    if (_i * ACT_CHAIN_NUM) % ACT_DEN < ACT_CHAIN_NUM:
        _pat[_i] = "A"
_ds = [_i for _i in range(ACT_DEN) if _pat[_i] == "D"]
for _j in range(min(AD_CHAIN_NUM, len(_ds))):
    _pat[_ds[(_j * len(_ds)) // max(AD_CHAIN_NUM, 1)]] = "AD"


def _chain_kind(p):
    return _pat[p % ACT_DEN]


def fp22_nearest(a):
    """Round fp32 to nearest 13-mantissa-bit value (FP22); the PE's f32r
    path truncates to FP22, so pre-rounded values pass through exactly."""
    a = np.ascontiguousarray(a, np.float32)
    return ((a.view(np.uint32) + np.uint32(0x200))
            & np.uint32(0xFFFFFC00)).view(np.float32)


def build_bass(repeat: int = 1):
    """Build the per-core Bass program (same NEFF on all 8 cores).

    repeat > 1 re-runs the whole main loop (idempotent writes) for timing.
    """
    nc = bacc.Bacc("TRN2", target_bir_lowering=False, debug=False,
                   num_devices=NCORES)
    xh_d = nc.dram_tensor("xh", [PAIRS, 128, B], FP16, kind="ExternalInput")
    xl8_d = nc.dram_tensor("xl8", [PAIRS, 128, B], F8E3, kind="ExternalInput")
    w3_d = nc.dram_tensor("w3", [128, PAIRS * WSTRIDE], W_DT,
                          kind="ExternalInput")
    negi_d = nc.dram_tensor("negi", [128, 128], FP16, kind="ExternalInput")
    # per pair 4 bias columns: bA=bsc+1536, bSA=2pi*(bsc+1536), bD=bsc,
    # bSD=2pi*bsc
    bb_d = nc.dram_tensor("bb", [128, PAIRS * 4], F32, kind="ExternalInput")
    yT_d = nc.dram_tensor("yT", [PAIRS, 128, B], OUT_DT, kind="ExternalOutput")

    with tile.TileContext(nc) as tc:
        with (
            tc.tile_pool(name="aux", bufs=1) as aux_pool,
            tc.tile_pool(name="wconst", bufs=1) as wc_pool,
            tc.tile_pool(name="xin", bufs=XB) as x_pool,
            tc.tile_pool(name="xlin", bufs=XB) as xl_pool,
            tc.tile_pool(name="oout", bufs=OB) as o_pool,
            tc.tile_pool(name="t2p", bufs=T_BUFS) as t2_pool,
            tc.tile_pool(name="tp", bufs=T_BUFS) as t_pool,
            tc.tile_pool(name="qp", bufs=T_BUFS) as q_pool,
            tc.tile_pool(name="ps", bufs=PSUM_BUFS, space="PSUM") as psum_pool,
        ):
            # --- constants (loaded once; scalar HWDGE ring + per-chunk
            # tiles so x loads and early pairs don't wait on the full
            # weight preload) ---
            WCH = 8                      # pairs per weight tile
            wc_tiles = []
            for _c in range(PAIRS // WCH):
                _w = WCH * WSTRIDE
                wct = wc_pool.tile([128, _w], W_DT, tag=f"w3_{_c}")
                (nc.scalar if os.environ.get("K_WQ", "scalar") == "scalar"
                 else nc.sync).dma_start(
                    wct[:], w3_d[:, _c * _w:(_c + 1) * _w])
                wc_tiles.append(wct)
            negi_sb = aux_pool.tile([128, 128], FP16)
            nc.sync.dma_start(negi_sb[:], negi_d[:])
            bb_sb = aux_pool.tile([128, PAIRS * 4], F32)
            nc.gpsimd.dma_start(bb_sb[:], bb_d[:])

            # --- main loop (optionally wrapped in a HW loop for timing) ---
            # One-tile software pipeline: the final (deferred) ops of tile i
            # (ident-matmul + Sin for ACT-chain, Sin for DVE-chain) are
            # emitted after tile i+1's matmuls, so the in-order PE/ACT
            # queues always have independent work ahead of a dependency
            # stall. The group's output DMA is emitted once its last Sin is.
            import contextlib
            rep_ctx = tc.For_i(0, repeat, 1) if repeat > 1 else contextlib.nullcontext()
            with rep_ctx:
                PEND_DEPTH = int(os.environ.get("K_PEND", "2"))
                pending = []       # deferred-op closures, oldest first

                def flush_one():
                    fn, grp = pending.pop(0)
                    fn()
                    grp[0] -= 1
                    if grp[0] == 0:
                        grp[1]()

                def flush_pending(limit=0):
                    while len(pending) > limit:
                        flush_one()

                for p0 in range(0, PAIRS, PB):
                    xt = x_pool.tile([128, PB * B], FP16, tag="xh")
                    nc.sync.dma_start(
                        xt[:].rearrange("p (a b) -> p a b", a=PB),
                        xh_d[p0:p0 + PB].rearrange("a p b -> p a b"))
                    xlt = xl_pool.tile([128, PB * B], F8E3, tag="xl8")
                    nc.sync.dma_start(
                        xlt[:].rearrange("p (a b) -> p a b", a=PB),
                        xl8_d[p0:p0 + PB].rearrange("a p b -> p a b"))
                    outt = o_pool.tile([128, PB * B], OUT_DT)

                    def _mk_dma(p0=p0, outt=outt):
                        def emit():
                            nc.scalar.dma_start(
                                yT_d[p0:p0 + PB].rearrange("a p b -> p a b"),
                                outt[:].rearrange("p (a b) -> p a b", a=PB))
                        return emit
                    grp = [PB * (B // EW), _mk_dma()]

                    for a in range(PB):
                        p = p0 + a
                        wc_sb = wc_tiles[p // WCH]
                        pw = (p % WCH) * WSTRIDE
                        if W32R:
                            passes = ((wc_sb[:, pw:pw + 128], xt),
                                      (wc_sb[:, pw + 128:pw + 256], xlt))
                        else:
                            passes = ((wc_sb[:, pw:pw + 128], xt),
                                      (wc_sb[:, pw + 128:pw + 256], xlt),
                                      (wc_sb[:, pw + 256:pw + 384], xt))
                        kind = _chain_kind(p)
                        act_chain = kind in ("A", "AD")
                        for e in range(B // EW):
                            v = psum_pool.tile([128, EW], F32)
                            for pi, (w_st, x_t) in enumerate(passes):
                                last = (pi == len(passes) - 1) \
                                    and not act_chain
                                for c in range(EW // MMW):
                                    lo = c * MMW
                                    bcol = a * B + e * EW + lo
                                    nc.tensor.matmul(
                                        v[:, lo:lo + MMW], w_st,
                                        x_t[:, bcol:bcol + MMW],
                                        start=(pi == 0), stop=last)
                            flush_pending(PEND_DEPTH - 1)
                            ob = outt[:, a * B + e * EW:a * B + (e + 1) * EW]
                            if act_chain:
                                bA = bb_sb[:, p * 4:p * 4 + 1]
                                bSA = bb_sb[:, p * 4 + 1:p * 4 + 2]
                                t2 = t2_pool.tile([128, EW], FP16)
                                if kind == "A":
                                    nc.scalar.activation(
                                        t2[:], v[:],
                                        mybir.ActivationFunctionType.Identity,
                                        bias=bA, scale=1.0)
                                else:   # "AD": round on DVE (fp16 out)
                                    nc.vector.tensor_scalar(
                                        t2[:], v[:], bA, None,
                                        AluOpType.add)

                                def _fin(v=v, t2=t2, ob=ob, bSA=bSA):
                                    for c in range(EW // MMW):
                                        lo = c * MMW
                                        nc.tensor.matmul(
                                            v[:, lo:lo + MMW], negi_sb[:],
                                            t2[:, lo:lo + MMW],
                                            start=False, stop=True)
                                    nc.scalar.activation(
                                        ob, v[:],
                                        mybir.ActivationFunctionType.Sin,
                                        bias=bSA, scale=TWO_PI)
                            else:
                                bD = bb_sb[:, p * 4 + 2:p * 4 + 3]
                                bSD = bb_sb[:, p * 4 + 3:p * 4 + 4]
                                t = t_pool.tile([128, EW], F32)
                                nc.vector.tensor_scalar(
                                    t[:], v[:], bD, MAGIC,
                                    AluOpType.add, AluOpType.add)
                                q = q_pool.tile([128, EW], F32)
                                nc.vector.scalar_tensor_tensor(
                                    q[:], t[:], MAGIC, v[:],
                                    op0=AluOpType.subtract,
                                    op1=AluOpType.subtract)

                                def _fin(q=q, ob=ob, bSD=bSD):
                                    nc.scalar.activation(
                                        ob, q[:],
                                        mybir.ActivationFunctionType.Sin,
                                        bias=bSD, scale=-TWO_PI)
                            pending.append((_fin, grp))
                flush_pending()
    nc.compile()
    return nc


def prep_inputs(x, weights, bias, omega):
    """Host-side layout prep -> list of 8 per-core input dicts."""
    x3 = x.reshape(B, NCORES, NS, D)
    # xT_all[c, n, i, b] = x[b, c*128+n, i]; blocked for cache friendliness
    xT_all = np.empty((NCORES, NS, D, B), np.float32)
    BBLK = 128
    for b0 in range(0, B, BBLK):
        xT_all[:, :, :, b0:b0 + BBLK] = x3[b0:b0 + BBLK].transpose(1, 2, 3, 0)

    xh_all = xT_all.astype(np.float16)
    xl8_all = ((xT_all - xh_all.astype(np.float32)) * np.float32(2048.0)
               ).astype(ml_dtypes.float8_e3m4)

    s1_full = (omega.astype(np.float32) * np.float32(INV_2PI))
    negi_host = (-np.eye(128)).astype(np.float16)
    in_maps = []
    for c in range(NCORES):
        sl = slice(c * NS, (c + 1) * NS)
        s1c = s1_full[sl]                      # [128]
        # prescale: W'[n,j,i] = W[n,j,i] * s1[n]
        wc = (weights[sl] * s1c[:, None, None]).astype(np.float32)
        wT = wc.transpose(0, 2, 1)             # [net, i, j]
        wbd = np.zeros((PAIRS, 128, 128), np.float32)
        wbd[:, :D, :D] = wT[0::2]
        wbd[:, D:, D:] = wT[1::2]
        if W32R:
            wpre = fp22_nearest(wbd).reshape(PAIRS, 128, 128)
            w2h = wpre * np.float32(2 ** -11)
            w3 = np.stack([wpre, w2h], axis=1)   # [PAIRS, 2, 128, 128]
        else:
            wh = wbd.astype(np.float16)
            wh2 = (wh.astype(np.float32) * np.float32(2 ** -11)
                   ).astype(np.float16)
            wl = (wbd - wh.astype(np.float32)).astype(np.float16)
            w3 = np.stack([wh, wh2, wl], axis=1)  # [PAIRS, 3, 128, 128]
        w3_host = np.ascontiguousarray(
            w3.transpose(2, 0, 1, 3).reshape(128, PAIRS * WSTRIDE))

        # bsc[part k, pair p]: rows 0-63 even net outputs, 64-127 odd
        bsc = (bias[sl].astype(np.float64) * s1c.astype(np.float64)[:, None])
        bp = np.zeros((128, PAIRS), np.float64)
        bp[:D, :] = bsc[0::2].T
        bp[D:, :] = bsc[1::2].T
        bb = np.zeros((128, PAIRS, 4), np.float64)
        bb[:, :, 0] = bp + C1536
        bb[:, :, 1] = TWO_PI * (bp + C1536)
        bb[:, :, 2] = bp
        bb[:, :, 3] = TWO_PI * bp
        bb_host = np.ascontiguousarray(
            bb.reshape(128, PAIRS * 4).astype(np.float32))

        in_maps.append({
            "xh": np.ascontiguousarray(xh_all[c].reshape(PAIRS, 128, B)),
            "xl8": np.ascontiguousarray(xl8_all[c].reshape(PAIRS, 128, B)),
            "w3": w3_host, "negi": negi_host, "bb": bb_host})
    return in_maps


def assemble_output(results):
    """[8 cores] of yT [PAIRS, 128, B] -> full [B, N*D] fp32."""
    out = np.empty((B, N * D), np.float32)
    for c in range(NCORES):
        yy = results[c]["yT"].reshape(NS * D, B)
        ov = out[:, c * NS * D:(c + 1) * NS * D]
        for b0 in range(0, B, 128):
            ov[b0:b0 + 128, :] = yy[:, b0:b0 + 128].T.astype(np.float32)
    return out


_NC_CACHE = {}


def kernel(x, weights, bias, omega):
    x = np.ascontiguousarray(x, np.float32)
    weights = np.ascontiguousarray(weights, np.float32)
    bias = np.ascontiguousarray(bias, np.float32)
    omega = np.ascontiguousarray(omega, np.float32)

    if "nc" not in _NC_CACHE:
        _NC_CACHE["nc"] = build_bass()
    nc = _NC_CACHE["nc"]
    in_maps = prep_inputs(x, weights, bias, omega)
    res = run_bass_kernel_spmd(nc, in_maps, core_ids=list(range(NCORES)))
    return assemble_output(res.results)
